# revision 47
# baseline (speedup 1.0000x reference)
"""Multi-head attention (B=4, T=2048, D=1024, H=16) on 8 TRN2 NeuronCores.

Sharding: core c handles batch b = c//2 and head-half hh = c%2 (8 heads,
512 of the 1024 channel dims). Each core computes its half of the head
outputs and a row-sharded output projection, producing a partial
[T, D] output. Host unshard: out[b] = partial[2b] + partial[2b+1]
+ b_o + b_v @ w_o.T (the value-bias contribution commutes through
attention because softmax rows sum to 1).

All matmuls in bf16 (tolerance 2e-2; bf16 with f32 psum accumulation
lands ~4e-3). Phase structure minimizes the serial prologue: K-proj
streams kb-outer behind its own DMA, attention starts right after the
first query-half of Q-proj, and the second Q-proj half plus the output
projection are injected at attention head boundaries. Output-tile DMAs
ride the SWDGE (gpsimd) ring so the softmax-denominator drain DMAs on
the SP ring never queue behind them.
"""

from contextlib import ExitStack

import numpy as np

import concourse.bass as bass
import concourse.mybir as mybir
import concourse.tile as tile
from concourse import bacc
from concourse.bass_utils import run_bass_kernel_spmd

B, T, D = 4, 2048, 1024
H = 16
DH = 64  # head dim
HALF = 512  # channels per core (8 heads)
N_CORES = 8

F32 = mybir.dt.float32
BF16 = mybir.dt.bfloat16

KB = 128  # contraction / partition block
NKB = D // KB  # 8
NJB = HALF // KB  # 4 j-blocks of the half
NTK = T // KB  # 16 key blocks
QH = 1024  # query half streamed per scores tile
NQH = T // QH  # 2


def build_kernel():
    nc = bacc.Bacc(
        "TRN2", target_bir_lowering=False, debug=False, num_devices=N_CORES
    )
    xqT = nc.dram_tensor("xqT", [D, T], BF16, kind="ExternalInput").ap()
    xkT = nc.dram_tensor("xkT", [D, T], BF16, kind="ExternalInput").ap()
    xvT = nc.dram_tensor("xvT", [D, T], BF16, kind="ExternalInput").ap()
    wqT = nc.dram_tensor("wqT", [D, HALF], BF16, kind="ExternalInput").ap()
    wkT = nc.dram_tensor("wkT", [D, HALF], BF16, kind="ExternalInput").ap()
    wvT = nc.dram_tensor("wvT", [D, HALF], BF16, kind="ExternalInput").ap()
    woT = nc.dram_tensor("woT", [HALF, D], BF16, kind="ExternalInput").ap()
    bq = nc.dram_tensor("bq", [HALF, 1], F32, kind="ExternalInput").ap()
    bk = nc.dram_tensor("bk", [HALF, 1], F32, kind="ExternalInput").ap()
    partial = nc.dram_tensor("partial", [T, D], BF16, kind="ExternalOutput").ap()

    with tile.TileContext(nc) as tc, ExitStack() as ctx:
        p_const = ctx.enter_context(tc.tile_pool(name="const", bufs=1))
        p_x = ctx.enter_context(tc.tile_pool(name="x", bufs=2))
        p_kt = ctx.enter_context(tc.tile_pool(name="kt", bufs=NJB))
        p_qt = ctx.enter_context(tc.tile_pool(name="qt", bufs=NJB))
        p_ot = ctx.enter_context(tc.tile_pool(name="ot", bufs=NJB))
        p_v = ctx.enter_context(tc.tile_pool(name="v", bufs=NTK))
        p_ex = ctx.enter_context(tc.tile_pool(name="ex", bufs=6))
        p_dr = ctx.enter_context(tc.tile_pool(name="dr", bufs=2))
        p_st = ctx.enter_context(tc.tile_pool(name="st", bufs=3))
        # PSUM: pool A = 2 bufs x 2 banks, pool B = 2 bufs x 2 banks.
        # Phase 1 K/Q-proj borrows both pools for 4 live accumulators;
        # attention uses A for scores / out-proj and B for AV.
        p_A = ctx.enter_context(tc.tile_pool(name="A", bufs=2, space="PSUM"))
        p_B = ctx.enter_context(tc.tile_pool(name="B", bufs=2, space="PSUM"))

        # ---- input DMAs, in consumption order (SP ring is FIFO).
        # Small constants ride the idle ACT ring so the SP ring starts
        # streaming xk immediately.
        # w_k rides the otherwise-empty ACT ring so xk-th0 leads the SP ring
        w_k = p_const.tile([KB, NKB, HALF], BF16, tag="wk")
        nc.scalar.dma_start(w_k[:], wkT.rearrange("(kb p) j -> p kb j", p=KB))
        xk = p_x.tile([KB, NKB, T], BF16, tag="x", name="xk")
        for th in range(NQH):
            for kb in range(NKB):
                nc.sync.dma_start(
                    xk[:, kb, th * QH : (th + 1) * QH],
                    xkT[kb * KB : (kb + 1) * KB, th * QH : (th + 1) * QH],
                )
            if th == 0:
                b_k = p_const.tile([KB, NJB], F32, tag="bk")
                nc.sync.dma_start(
                    b_k[:], bk.rearrange("(jb p) one -> p (jb one)", p=KB)
                )
        w_q = p_const.tile([KB, NKB, HALF], BF16, tag="wq")
        nc.sync.dma_start(w_q[:], wqT.rearrange("(kb p) j -> p kb j", p=KB))
        xq = p_x.tile([KB, NKB, T], BF16, tag="x", name="xq")
        for th in range(NQH):
            for kb in range(NKB):
                nc.sync.dma_start(
                    xq[:, kb, th * QH : (th + 1) * QH],
                    xqT[kb * KB : (kb + 1) * KB, th * QH : (th + 1) * QH],
                )
            if th == 0:
                b_q = p_const.tile([KB, NJB], F32, tag="bq")
                nc.sync.dma_start(
                    b_q[:], bq.rearrange("(jb p) one -> p (jb one)", p=KB)
                )
        w_v = p_const.tile([KB, NKB, HALF], BF16, tag="wv")
        nc.sync.dma_start(w_v[:], wvT.rearrange("(kb p) j -> p kb j", p=KB))

        # ---- persistent sbuf tiles ----
        ones_bc = p_const.tile([DH + 1, DH], BF16, tag="ones_bc")
        nc.vector.memset(ones_bc[:], 1.0)
        kt_tiles = [p_kt.tile([KB, T], BF16, tag="kt", name=f"kt{j}") for j in range(NJB)]
        qt_tiles = [p_qt.tile([KB, T], BF16, tag="qt", name=f"qt{j}") for j in range(NJB)]
        ot_tiles = [p_ot.tile([KB, T], BF16, tag="ot", name=f"ot{j}") for j in range(NJB)]
        v_tiles = [
            p_v.tile([KB, H // 2, DH + 1], BF16, tag="v", name=f"v{t}")
            for t in range(NTK)
        ]
        for t in range(NTK):
            nc.vector.memset(v_tiles[t][:, :, DH : DH + 1], 1.0)

        # xv reuses the xk buffer once K-proj is done with it
        xv = p_x.tile([KB, NKB, T], BF16, tag="x", name="xv")
        for th in range(NQH):
            for kb in range(NKB):
                nc.sync.dma_start(
                    xv[:, kb, th * QH : (th + 1) * QH],
                    xvT[kb * KB : (kb + 1) * KB, th * QH : (th + 1) * QH],
                )
        w_o = p_const.tile([KB, NJB, D], BF16, tag="wo")
        nc.sync.dma_start(w_o[:], woT.rearrange("(jb p) n -> p jb n", p=KB))

        # ---- K^T projection, kb-outer so it streams behind the xk DMA ----
        # 4 live accumulators [128, 1024] = all 8 psum banks (pools A+B)
        for th in range(NQH):
            ps = [
                (p_A if jb < 2 else p_B).tile(
                    [KB, QH], F32, tag=("mm" if jb < 2 else "av"), name=f"kp{jb}"
                )
                for jb in range(NJB)
            ]
            for kb in range(NKB):
                for jb in range(NJB):
                    for s in range(2):
                        nc.tensor.matmul(
                            ps[jb][:, s * 512 : (s + 1) * 512],
                            w_k[:, kb, jb * KB : (jb + 1) * KB],
                            xk[:, kb, th * QH + s * 512 : th * QH + (s + 1) * 512],
                            start=(kb == 0),
                            stop=(kb == NKB - 1),
                        )
            for jb in range(NJB):
                nc.vector.tensor_scalar_add(
                    kt_tiles[jb][:, th * QH : (th + 1) * QH],
                    ps[jb][:],
                    b_k[:, jb : jb + 1],
                )

        # ---- V projection one t-block (natural layout): v[t] = [128 t, 8 h, 65] ----
        def emit_vproj(tb):
            ps = p_A.tile([KB, HALF], F32, tag="mm", name="vp")
            for kb in range(NKB):
                nc.tensor.matmul(
                    ps[:],
                    xv[:, kb, tb * KB : (tb + 1) * KB],
                    w_v[:, kb, :],
                    start=(kb == 0),
                    stop=(kb == NKB - 1),
                )
            nc.vector.tensor_copy(
                v_tiles[tb][:, :, 0:DH], ps.rearrange("p (h d) -> p h d", d=DH)
            )

        # ---- Q^T projection for one query-half (kb-inner, jb groups) ----
        def emit_qproj(jb, th):
            ps = p_A.tile([KB, QH], F32, tag="mm", name="qp")
            for kb in range(NKB):
                for s in range(2):
                    nc.tensor.matmul(
                        ps[:, s * 512 : (s + 1) * 512],
                        w_q[:, kb, jb * KB : (jb + 1) * KB],
                        xq[:, kb, th * QH + s * 512 : th * QH + (s + 1) * 512],
                        start=(kb == 0),
                        stop=(kb == NKB - 1),
                    )
            nc.vector.tensor_scalar_add(
                qt_tiles[jb][:, th * QH : (th + 1) * QH], ps[:], b_q[:, jb : jb + 1]
            )

        def emit_qproj_half(jb, th, ch):
            # half-width (one 512-col chunk): short psum residency, so it
            # slots between attention score tiles with minimal ACT stall
            c0 = th * QH + ch * 512
            ps = p_A.tile([KB, 512], F32, tag="mm", name="qph")
            for kb in range(NKB):
                nc.tensor.matmul(
                    ps[:],
                    w_q[:, kb, jb * KB : (jb + 1) * KB],
                    xq[:, kb, c0 : c0 + 512],
                    start=(kb == 0),
                    stop=(kb == NKB - 1),
                )
            nc.vector.tensor_scalar_add(
                qt_tiles[jb][:, c0 : c0 + 512], ps[:], b_q[:, jb : jb + 1]
            )

        for jb in range(NJB):
            emit_qproj(jb, 0)

        # ---- attention with interleaved Q-th1 / out-projection ----
        def emit_scores(h, qh, tk):
            jp, hi = h // 2, h % 2
            sc = p_A.tile([KB, QH], F32, tag="mm", name="sc")
            for s in range(2):
                nc.tensor.matmul(
                    sc[:, s * 512 : (s + 1) * 512],
                    kt_tiles[jp][hi * DH : (hi + 1) * DH, tk * KB : (tk + 1) * KB],
                    qt_tiles[jp][
                        hi * DH : (hi + 1) * DH,
                        qh * QH + s * 512 : qh * QH + (s + 1) * 512,
                    ],
                    start=True,
                    stop=True,
                )
            ex = p_ex.tile([KB, QH], BF16, tag="ex")
            nc.scalar.activation(
                ex[:], sc[:], mybir.ActivationFunctionType.Exp, scale=0.125
            )
            return ex

        def emit_av(h, qh, tk, ex, av):
            for s in range(2):
                nc.tensor.matmul(
                    av[:, s * 512 : (s + 1) * 512],
                    v_tiles[tk][:, h, :],
                    ex[:, s * 512 : (s + 1) * 512],
                    start=(tk == 0),
                    stop=(tk == NTK - 1),
                )

        def emit_drain(h, qh, av, pe_bcast=False):
            jp, hi = h // 2, h % 2
            dsb = p_dr.tile([DH + 1, QH], BF16, tag="dsb")
            nc.vector.tensor_copy(dsb[DH : DH + 1, :], av[DH : DH + 1, :])
            rc = p_dr.tile([DH, QH], F32, tag="rc")
            if pe_bcast:
                # low-latency path: broadcast the denominator row across
                # partitions with a rank-1 matmul on the PE (no DMA)
                bcp = p_A.tile([DH, QH], F32, tag="mm", name="bcp")
                for s in range(2):
                    nc.tensor.matmul(
                        bcp[:, s * 512 : (s + 1) * 512],
                        ones_bc[DH : DH + 1, :],
                        dsb[DH : DH + 1, s * 512 : (s + 1) * 512],
                        start=True,
                        stop=True,
                    )
                nc.vector.reciprocal_approx_fast(rc[:], bcp[:])
            else:
                bc = p_dr.tile([DH, QH], BF16, tag="bc")
                nc.sync.dma_start(
                    bc[:], dsb[DH : DH + 1, None, :].broadcast_to([1, DH, QH])
                )
                bcf = p_dr.tile([DH, QH], F32, tag="bcf")
                nc.vector.tensor_copy(bcf[:], bc[:])
                nc.vector.reciprocal_approx_fast(rc[:], bcf[:])
            dst = ot_tiles[jp][hi * DH : (hi + 1) * DH, qh * QH : (qh + 1) * QH]
            if hi == 0:
                nc.vector.tensor_mul(dst, av[0:DH, :], rc[:])
            else:
                stg = p_dr.tile([DH, QH], BF16, tag="stg")
                nc.vector.tensor_mul(stg[:], av[0:DH, :], rc[:])
                nc.sync.dma_start(dst, stg[:])

        def emit_outproj_mm(po, tblk, jps):
            for jp in jps:
                for s in range(2):
                    nc.tensor.matmul(
                        po[:, s * 512 : (s + 1) * 512],
                        ot_tiles[jp][:, tblk * KB : (tblk + 1) * KB],
                        w_o[:, jp, s * 512 : (s + 1) * 512],
                        start=(jp == 0),
                        stop=(jp == NJB - 1),
                    )

        def emit_outproj_st(po, tblk, use_act=False):
            st = p_st.tile([KB, D], BF16, tag="st")
            if use_act:
                nc.scalar.copy(st[:], po[:])
            else:
                nc.vector.tensor_copy(st[:], po[:])
            nc.sync.dma_start(partial[tblk * KB : (tblk + 1) * KB, :], st[:])

        def emit_outproj(tblk):
            po = p_A.tile([KB, D], F32, tag="mm", name="po")
            emit_outproj_mm(po, tblk, range(NJB))
            emit_outproj_st(po, tblk)

        def emit_outproj_half(tblk, nh):
            # half-width injection: short psum residency between score tiles
            po = p_A.tile([KB, 512], F32, tag="mm", name="poh")
            for jp in range(NJB):
                nc.tensor.matmul(
                    po[:],
                    ot_tiles[jp][:, tblk * KB : (tblk + 1) * KB],
                    w_o[:, jp, nh * 512 : (nh + 1) * 512],
                    start=(jp == 0),
                    stop=(jp == NJB - 1),
                )
            st = p_st.tile([KB, 512], BF16, tag="st")
            nc.vector.tensor_copy(st[:], po[:])
            nc.sync.dma_start(
                partial[tblk * KB : (tblk + 1) * KB, nh * 512 : (nh + 1) * 512],
                st[:],
            )

        pending_av = None  # (h, qh, tk, ex, av)
        # qh1 processes even heads last: their drains skip the partition-
        # shift DMA, shortening the critical chain into the tail out-proj
        qh1_order = [1, 3, 5, 7, 0, 2, 4, 6]
        for qh in range(NQH):
            heads = range(H // 2) if qh == 0 else qh1_order
            for idx, h in enumerate(heads):
                av = p_B.tile([DH + 1, QH], F32, tag="av", name="av")
                for tk in range(NTK):
                    ex = emit_scores(h, qh, tk)
                    # V-projection streams inside the very first head: AV of
                    # tile tk only needs v[tk], emitted one iteration earlier
                    if qh == 0 and idx == 0:
                        emit_vproj(tk)
                    # deferred Q-proj right after the head's first scores so
                    # the exp of tile 0 covers part of its PE time
                    if qh == 0 and 1 <= idx <= 7 and tk == 0:
                        emit_qproj_half((idx - 1) // 2, 1, (idx - 1) % 2)
                    if qh == 0 and idx == 7 and tk == 8:
                        emit_qproj_half(3, 1, 1)
                    # first qh1 head waits for the last qh0 drain chain, so
                    # its injections sit later in the tk loop
                    if qh == 1 and tk == (10 if idx == 0 else 4):
                        emit_outproj_half(idx, 0)
                    if qh == 1 and tk == (14 if idx == 0 else 12):
                        emit_outproj_half(idx, 1)
                    if pending_av is not None:
                        ph, pqh, ptk, pex, pav = pending_av
                        emit_av(ph, pqh, ptk, pex, pav)
                        if ptk == NTK - 1:
                            emit_drain(ph, pqh, pav)
                    pending_av = (h, qh, tk, ex, av)
        ph, pqh, ptk, pex, pav = pending_av
        emit_av(ph, pqh, ptk, pex, pav)
        # final drain: PE-broadcast path, no DMA on the critical chain
        emit_drain(ph, pqh, pav, pe_bcast=True)
        # tail: 4-deep software pipeline (pool A + freed pool-B AV slots) so
        # the jp0-2 matmuls (whose ot rows drained long ago) run while the
        # last head's drain completes, and the PE stream stays dense
        window = []
        for i, t in enumerate(range(8, NTK)):
            pool, tag = (p_A, "mm") if i % 2 == 0 else (p_B, "av")
            po = pool.tile([KB, D], F32, tag=tag, name="po")
            emit_outproj_mm(po, t, range(NJB - 1))
            window.append((t, po))
            if len(window) == 4:
                pt, ppo = window.pop(0)
                emit_outproj_mm(ppo, pt, [NJB - 1])
                emit_outproj_st(ppo, pt, use_act=(pt % 2 == 1))
        for pt, ppo in window:
            emit_outproj_mm(ppo, pt, [NJB - 1])
            emit_outproj_st(ppo, pt, use_act=(pt % 2 == 1))

    nc.compile()
    return nc


def kernel(**inputs: np.ndarray) -> np.ndarray:
    import ml_dtypes

    BF = ml_dtypes.bfloat16

    query = np.asarray(inputs["query"], dtype=np.float32)
    key = np.asarray(inputs["key"], dtype=np.float32)
    value = np.asarray(inputs["value"], dtype=np.float32)
    w_q = np.asarray(inputs["w_q"], dtype=np.float32)
    b_q = np.asarray(inputs["b_q"], dtype=np.float32)
    w_k = np.asarray(inputs["w_k"], dtype=np.float32)
    b_k = np.asarray(inputs["b_k"], dtype=np.float32)
    w_v = np.asarray(inputs["w_v"], dtype=np.float32)
    b_v = np.asarray(inputs["b_v"], dtype=np.float32)
    w_o = np.asarray(inputs["w_o"], dtype=np.float32)
    b_o = np.asarray(inputs["b_o"], dtype=np.float32)

    nc = build_kernel()

    in_maps = []
    for c in range(N_CORES):
        b = c // 2
        hh = c % 2
        sl = slice(hh * HALF, (hh + 1) * HALF)
        in_maps.append(
            {
                "xqT": np.ascontiguousarray(query[b].T).astype(BF),
                "xkT": np.ascontiguousarray(key[b].T).astype(BF),
                "xvT": np.ascontiguousarray(value[b].T).astype(BF),
                "wqT": np.ascontiguousarray(w_q[sl, :].T).astype(BF),
                "wkT": np.ascontiguousarray(w_k[sl, :].T).astype(BF),
                "wvT": np.ascontiguousarray(w_v[sl, :].T).astype(BF),
                "woT": np.ascontiguousarray(w_o[:, sl].T).astype(BF),
                "bq": np.ascontiguousarray(b_q[sl].reshape(HALF, 1)),
                "bk": np.ascontiguousarray(b_k[sl].reshape(HALF, 1)),
            }
        )

    res = run_bass_kernel_spmd(nc, in_maps, core_ids=list(range(N_CORES)))

    const_row = (b_v[None, :] @ w_o.T + b_o[None, :]).astype(np.float32)
    out = np.empty((B, T, D), dtype=np.float32)
    for b in range(B):
        out[b] = np.asarray(res.results[2 * b]["partial"], dtype=np.float32)
        out[b] += np.asarray(res.results[2 * b + 1]["partial"], dtype=np.float32)
        out[b] += const_row
    return out


# revision 48
# speedup vs baseline: 1.1302x; 1.1302x over previous
"""Multi-head attention (B=4, T=2048, D=1024, H=16) on 8 TRN2 NeuronCores.

Sharding: core c handles batch b = c//2 and head-half hh = c%2 (8 heads,
512 of the 1024 channel dims). Each core computes its half of the head
outputs and a row-sharded output projection, producing a partial
[T, D] output. Host unshard: out[b] = partial[2b] + partial[2b+1]
+ b_o + b_v @ w_o.T (the value-bias contribution commutes through
attention because softmax rows sum to 1).

All matmuls in bf16 (tolerance 2e-2; bf16 with f32 psum accumulation
lands ~4e-3). Phase structure minimizes the serial prologue: K-proj
streams kb-outer behind its own DMA, attention starts right after the
first query-half of Q-proj, and the second Q-proj half plus the output
projection are injected at attention head boundaries. Output-tile DMAs
ride the SWDGE (gpsimd) ring so the softmax-denominator drain DMAs on
the SP ring never queue behind them.
"""

from contextlib import ExitStack

import numpy as np

import concourse.bass as bass
import concourse.mybir as mybir
import concourse.tile as tile
from concourse import bacc
from concourse.bass_utils import run_bass_kernel_spmd

B, T, D = 4, 2048, 1024
H = 16
DH = 64  # head dim
HALF = 512  # channels per core (8 heads)
N_CORES = 8

F32 = mybir.dt.float32
BF16 = mybir.dt.bfloat16

KB = 128  # contraction / partition block
NKB = D // KB  # 8
NJB = HALF // KB  # 4 j-blocks of the half
NTK = T // KB  # 16 key blocks
QH = 1024  # query half streamed per scores tile
NQH = T // QH  # 2


def build_kernel():
    nc = bacc.Bacc(
        "TRN2", target_bir_lowering=False, debug=False, num_devices=N_CORES
    )
    xqT = nc.dram_tensor("xqT", [D, T], BF16, kind="ExternalInput").ap()
    xkT = nc.dram_tensor("xkT", [D, T], BF16, kind="ExternalInput").ap()
    xvT = nc.dram_tensor("xvT", [D, T], BF16, kind="ExternalInput").ap()
    wqT = nc.dram_tensor("wqT", [D, HALF], BF16, kind="ExternalInput").ap()
    wkT = nc.dram_tensor("wkT", [D, HALF], BF16, kind="ExternalInput").ap()
    wvT = nc.dram_tensor("wvT", [D, HALF], BF16, kind="ExternalInput").ap()
    woT = nc.dram_tensor("woT", [HALF, D], BF16, kind="ExternalInput").ap()
    bq = nc.dram_tensor("bq", [HALF, 1], F32, kind="ExternalInput").ap()
    bk = nc.dram_tensor("bk", [HALF, 1], F32, kind="ExternalInput").ap()
    partial = nc.dram_tensor("partial", [T, D], BF16, kind="ExternalOutput").ap()

    with tile.TileContext(nc) as tc, ExitStack() as ctx:
        p_const = ctx.enter_context(tc.tile_pool(name="const", bufs=1))
        p_x = ctx.enter_context(tc.tile_pool(name="x", bufs=2))
        p_kt = ctx.enter_context(tc.tile_pool(name="kt", bufs=NJB))
        p_qt = ctx.enter_context(tc.tile_pool(name="qt", bufs=NJB))
        p_ot = ctx.enter_context(tc.tile_pool(name="ot", bufs=NJB))
        p_v = ctx.enter_context(tc.tile_pool(name="v", bufs=NTK))
        p_ex = ctx.enter_context(tc.tile_pool(name="ex", bufs=4))
        p_dr = ctx.enter_context(tc.tile_pool(name="dr", bufs=2))
        p_st = ctx.enter_context(tc.tile_pool(name="st", bufs=2))
        # PSUM: pool A = 2 bufs x 2 banks, pool B = 2 bufs x 2 banks.
        # Phase 1 K/Q-proj borrows both pools for 4 live accumulators;
        # attention uses A for scores / out-proj and B for AV.
        p_A = ctx.enter_context(tc.tile_pool(name="A", bufs=2, space="PSUM"))
        p_B = ctx.enter_context(tc.tile_pool(name="B", bufs=2, space="PSUM"))

        # ---- input DMAs, in consumption order (SP ring is FIFO).
        # Small constants ride the idle ACT ring so the SP ring starts
        # streaming xk immediately.
        # w_k rides the otherwise-empty ACT ring so xk-th0 leads the SP ring
        w_k = p_const.tile([KB, NKB, HALF], BF16, tag="wk")
        nc.scalar.dma_start(w_k[:], wkT.rearrange("(kb p) j -> p kb j", p=KB))
        xk = p_x.tile([KB, NKB, T], BF16, tag="x", name="xk")
        for th in range(NQH):
            for kb in range(NKB):
                nc.sync.dma_start(
                    xk[:, kb, th * QH : (th + 1) * QH],
                    xkT[kb * KB : (kb + 1) * KB, th * QH : (th + 1) * QH],
                )
            if th == 0:
                b_k = p_const.tile([KB, NJB], F32, tag="bk")
                nc.sync.dma_start(
                    b_k[:], bk.rearrange("(jb p) one -> p (jb one)", p=KB)
                )
        w_q = p_const.tile([KB, NKB, HALF], BF16, tag="wq")
        nc.sync.dma_start(w_q[:], wqT.rearrange("(kb p) j -> p kb j", p=KB))
        xq = p_x.tile([KB, NKB, T], BF16, tag="x", name="xq")
        for th in range(NQH):
            for kb in range(NKB):
                nc.sync.dma_start(
                    xq[:, kb, th * QH : (th + 1) * QH],
                    xqT[kb * KB : (kb + 1) * KB, th * QH : (th + 1) * QH],
                )
            if th == 0:
                b_q = p_const.tile([KB, NJB], F32, tag="bq")
                nc.sync.dma_start(
                    b_q[:], bq.rearrange("(jb p) one -> p (jb one)", p=KB)
                )
        w_v = p_const.tile([KB, NKB, HALF], BF16, tag="wv")
        nc.sync.dma_start(w_v[:], wvT.rearrange("(kb p) j -> p kb j", p=KB))

        # ---- persistent sbuf tiles ----
        ones_bc = p_const.tile([DH + 1, DH], BF16, tag="ones_bc")
        nc.vector.memset(ones_bc[:], 1.0)
        kt_tiles = [p_kt.tile([KB, T], BF16, tag="kt", name=f"kt{j}") for j in range(NJB)]
        qt_tiles = [p_qt.tile([KB, T], BF16, tag="qt", name=f"qt{j}") for j in range(NJB)]
        ot_tiles = [p_ot.tile([KB, T], BF16, tag="ot", name=f"ot{j}") for j in range(NJB)]
        v_tiles = [
            p_v.tile([KB, H // 2, DH + 1], BF16, tag="v", name=f"v{t}")
            for t in range(NTK)
        ]
        for t in range(NTK):
            nc.vector.memset(v_tiles[t][:, :, DH : DH + 1], 1.0)

        # xv reuses the xk buffer once K-proj is done with it
        xv = p_x.tile([KB, NKB, T], BF16, tag="x", name="xv")
        for th in range(NQH):
            for kb in range(NKB):
                nc.sync.dma_start(
                    xv[:, kb, th * QH : (th + 1) * QH],
                    xvT[kb * KB : (kb + 1) * KB, th * QH : (th + 1) * QH],
                )
        w_o = p_const.tile([KB, NJB, D], BF16, tag="wo")
        nc.sync.dma_start(w_o[:], woT.rearrange("(jb p) n -> p jb n", p=KB))

        # ---- K^T projection, kb-outer so it streams behind the xk DMA ----
        # 4 live accumulators [128, 1024] = all 8 psum banks (pools A+B)
        for th in range(NQH):
            ps = [
                (p_A if jb < 2 else p_B).tile(
                    [KB, QH], F32, tag=("mm" if jb < 2 else "av"), name=f"kp{jb}"
                )
                for jb in range(NJB)
            ]
            for kb in range(NKB):
                for jb in range(NJB):
                    for s in range(2):
                        nc.tensor.matmul(
                            ps[jb][:, s * 512 : (s + 1) * 512],
                            w_k[:, kb, jb * KB : (jb + 1) * KB],
                            xk[:, kb, th * QH + s * 512 : th * QH + (s + 1) * 512],
                            start=(kb == 0),
                            stop=(kb == NKB - 1),
                        )
            for jb in range(NJB):
                nc.vector.tensor_scalar_add(
                    kt_tiles[jb][:, th * QH : (th + 1) * QH],
                    ps[jb][:],
                    b_k[:, jb : jb + 1],
                )

        # ---- V projection one t-block (natural layout): v[t] = [128 t, 8 h, 65] ----
        def emit_vproj(tb):
            ps = p_A.tile([KB, HALF], F32, tag="mm", name="vp")
            for kb in range(NKB):
                nc.tensor.matmul(
                    ps[:],
                    xv[:, kb, tb * KB : (tb + 1) * KB],
                    w_v[:, kb, :],
                    start=(kb == 0),
                    stop=(kb == NKB - 1),
                )
            nc.vector.tensor_copy(
                v_tiles[tb][:, :, 0:DH], ps.rearrange("p (h d) -> p h d", d=DH)
            )

        # ---- Q^T projection for one query-half (kb-inner, jb groups) ----
        def emit_qproj(jb, th):
            ps = p_A.tile([KB, QH], F32, tag="mm", name="qp")
            for kb in range(NKB):
                for s in range(2):
                    nc.tensor.matmul(
                        ps[:, s * 512 : (s + 1) * 512],
                        w_q[:, kb, jb * KB : (jb + 1) * KB],
                        xq[:, kb, th * QH + s * 512 : th * QH + (s + 1) * 512],
                        start=(kb == 0),
                        stop=(kb == NKB - 1),
                    )
            nc.vector.tensor_scalar_add(
                qt_tiles[jb][:, th * QH : (th + 1) * QH], ps[:], b_q[:, jb : jb + 1]
            )

        def emit_qproj_half(jb, th, ch):
            # half-width (one 512-col chunk): short psum residency, so it
            # slots between attention score tiles with minimal ACT stall
            c0 = th * QH + ch * 512
            ps = p_A.tile([KB, 512], F32, tag="mm", name="qph")
            for kb in range(NKB):
                nc.tensor.matmul(
                    ps[:],
                    w_q[:, kb, jb * KB : (jb + 1) * KB],
                    xq[:, kb, c0 : c0 + 512],
                    start=(kb == 0),
                    stop=(kb == NKB - 1),
                )
            nc.vector.tensor_scalar_add(
                qt_tiles[jb][:, c0 : c0 + 512], ps[:], b_q[:, jb : jb + 1]
            )

        for jb in range(NJB):
            emit_qproj(jb, 0)

        # ---- attention with interleaved Q-th1 / out-projection ----
        def emit_scores(h, qh, tk):
            jp, hi = h // 2, h % 2
            sc = p_A.tile([KB, QH], F32, tag="mm", name="sc")
            for s in range(2):
                nc.tensor.matmul(
                    sc[:, s * 512 : (s + 1) * 512],
                    kt_tiles[jp][hi * DH : (hi + 1) * DH, tk * KB : (tk + 1) * KB],
                    qt_tiles[jp][
                        hi * DH : (hi + 1) * DH,
                        qh * QH + s * 512 : qh * QH + (s + 1) * 512,
                    ],
                    start=True,
                    stop=True,
                )
            ex = p_ex.tile([KB, QH], BF16, tag="ex")
            nc.scalar.activation(
                ex[:], sc[:], mybir.ActivationFunctionType.Exp, scale=0.125
            )
            return ex

        def emit_av(h, qh, tk, ex, av):
            for s in range(2):
                nc.tensor.matmul(
                    av[:, s * 512 : (s + 1) * 512],
                    v_tiles[tk][:, h, :],
                    ex[:, s * 512 : (s + 1) * 512],
                    start=(tk == 0),
                    stop=(tk == NTK - 1),
                )

        def emit_drain(h, qh, av, pe_bcast=False):
            jp, hi = h // 2, h % 2
            dsb = p_dr.tile([DH + 1, QH], BF16, tag="dsb")
            nc.vector.tensor_copy(dsb[DH : DH + 1, :], av[DH : DH + 1, :])
            rc = p_dr.tile([DH, QH], F32, tag="rc")
            if pe_bcast:
                # low-latency path: broadcast the denominator row across
                # partitions with a rank-1 matmul on the PE (no DMA)
                bcp = p_A.tile([DH, QH], F32, tag="mm", name="bcp")
                for s in range(2):
                    nc.tensor.matmul(
                        bcp[:, s * 512 : (s + 1) * 512],
                        ones_bc[DH : DH + 1, :],
                        dsb[DH : DH + 1, s * 512 : (s + 1) * 512],
                        start=True,
                        stop=True,
                    )
                nc.vector.reciprocal_approx_fast(rc[:], bcp[:])
            else:
                bc = p_dr.tile([DH, QH], BF16, tag="bc")
                nc.sync.dma_start(
                    bc[:], dsb[DH : DH + 1, None, :].broadcast_to([1, DH, QH])
                )
                bcf = p_dr.tile([DH, QH], F32, tag="bcf")
                nc.vector.tensor_copy(bcf[:], bc[:])
                nc.vector.reciprocal_approx_fast(rc[:], bcf[:])
            dst = ot_tiles[jp][hi * DH : (hi + 1) * DH, qh * QH : (qh + 1) * QH]
            if hi == 0:
                nc.vector.tensor_mul(dst, av[0:DH, :], rc[:])
            else:
                stg = p_dr.tile([DH, QH], BF16, tag="stg")
                nc.vector.tensor_mul(stg[:], av[0:DH, :], rc[:])
                nc.sync.dma_start(dst, stg[:])

        def emit_outproj_mm(po, tblk, jps):
            for jp in jps:
                for s in range(2):
                    nc.tensor.matmul(
                        po[:, s * 512 : (s + 1) * 512],
                        ot_tiles[jp][:, tblk * KB : (tblk + 1) * KB],
                        w_o[:, jp, s * 512 : (s + 1) * 512],
                        start=(jp == 0),
                        stop=(jp == NJB - 1),
                    )

        def emit_outproj_st(po, tblk, use_act=False):
            st = p_st.tile([KB, D], BF16, tag="st")
            if use_act:
                nc.scalar.copy(st[:], po[:])
            else:
                nc.vector.tensor_copy(st[:], po[:])
            nc.sync.dma_start(partial[tblk * KB : (tblk + 1) * KB, :], st[:])

        def emit_outproj(tblk):
            po = p_A.tile([KB, D], F32, tag="mm", name="po")
            emit_outproj_mm(po, tblk, range(NJB))
            emit_outproj_st(po, tblk)

        def emit_outproj_half(tblk, nh):
            # half-width injection: short psum residency between score tiles
            po = p_A.tile([KB, 512], F32, tag="mm", name="poh")
            for jp in range(NJB):
                nc.tensor.matmul(
                    po[:],
                    ot_tiles[jp][:, tblk * KB : (tblk + 1) * KB],
                    w_o[:, jp, nh * 512 : (nh + 1) * 512],
                    start=(jp == 0),
                    stop=(jp == NJB - 1),
                )
            st = p_st.tile([KB, 512], BF16, tag="st")
            nc.vector.tensor_copy(st[:], po[:])
            nc.sync.dma_start(
                partial[tblk * KB : (tblk + 1) * KB, nh * 512 : (nh + 1) * 512],
                st[:],
            )

        pending_av = None  # (h, qh, tk, ex, av)
        # qh1 processes even heads last: their drains skip the partition-
        # shift DMA, shortening the critical chain into the tail out-proj
        qh1_order = [1, 3, 5, 7, 0, 2, 4, 6]
        for qh in range(NQH):
            heads = range(H // 2) if qh == 0 else qh1_order
            for idx, h in enumerate(heads):
                av = p_B.tile([DH + 1, QH], F32, tag="av", name="av")
                for tk in range(NTK):
                    ex = emit_scores(h, qh, tk)
                    # V-projection streams inside the very first head: AV of
                    # tile tk only needs v[tk], emitted one iteration earlier
                    if qh == 0 and idx == 0:
                        emit_vproj(tk)
                    # deferred Q-proj right after the head's first scores so
                    # the exp of tile 0 covers part of its PE time
                    if qh == 0 and 1 <= idx <= 7 and tk == 0:
                        emit_qproj_half((idx - 1) // 2, 1, (idx - 1) % 2)
                    if qh == 0 and idx == 7 and tk == 8:
                        emit_qproj_half(3, 1, 1)
                    if pending_av is not None:
                        ph, pqh, ptk, pex, pav = pending_av
                        emit_av(ph, pqh, ptk, pex, pav)
                        if ptk == NTK - 1:
                            emit_drain(ph, pqh, pav)
                    pending_av = (h, qh, tk, ex, av)
                if qh == 1:
                    emit_outproj(idx)
        ph, pqh, ptk, pex, pav = pending_av
        emit_av(ph, pqh, ptk, pex, pav)
        # final drain: PE-broadcast path, no DMA on the critical chain
        emit_drain(ph, pqh, pav, pe_bcast=True)
        # tail: 4-deep software pipeline (pool A + freed pool-B AV slots) so
        # the jp0-2 matmuls (whose ot rows drained long ago) run while the
        # last head's drain completes, and the PE stream stays dense
        window = []
        for i, t in enumerate(range(8, NTK)):
            pool, tag = (p_A, "mm") if i % 2 == 0 else (p_B, "av")
            po = pool.tile([KB, D], F32, tag=tag, name="po")
            emit_outproj_mm(po, t, range(NJB - 1))
            window.append((t, po))
            if len(window) == 4:
                pt, ppo = window.pop(0)
                emit_outproj_mm(ppo, pt, [NJB - 1])
                emit_outproj_st(ppo, pt, use_act=(pt % 2 == 1))
        for pt, ppo in window:
            emit_outproj_mm(ppo, pt, [NJB - 1])
            emit_outproj_st(ppo, pt, use_act=(pt % 2 == 1))

    nc.compile()
    return nc


def kernel(**inputs: np.ndarray) -> np.ndarray:
    import ml_dtypes

    BF = ml_dtypes.bfloat16

    query = np.asarray(inputs["query"], dtype=np.float32)
    key = np.asarray(inputs["key"], dtype=np.float32)
    value = np.asarray(inputs["value"], dtype=np.float32)
    w_q = np.asarray(inputs["w_q"], dtype=np.float32)
    b_q = np.asarray(inputs["b_q"], dtype=np.float32)
    w_k = np.asarray(inputs["w_k"], dtype=np.float32)
    b_k = np.asarray(inputs["b_k"], dtype=np.float32)
    w_v = np.asarray(inputs["w_v"], dtype=np.float32)
    b_v = np.asarray(inputs["b_v"], dtype=np.float32)
    w_o = np.asarray(inputs["w_o"], dtype=np.float32)
    b_o = np.asarray(inputs["b_o"], dtype=np.float32)

    nc = build_kernel()

    in_maps = []
    for c in range(N_CORES):
        b = c // 2
        hh = c % 2
        sl = slice(hh * HALF, (hh + 1) * HALF)
        in_maps.append(
            {
                "xqT": np.ascontiguousarray(query[b].T).astype(BF),
                "xkT": np.ascontiguousarray(key[b].T).astype(BF),
                "xvT": np.ascontiguousarray(value[b].T).astype(BF),
                "wqT": np.ascontiguousarray(w_q[sl, :].T).astype(BF),
                "wkT": np.ascontiguousarray(w_k[sl, :].T).astype(BF),
                "wvT": np.ascontiguousarray(w_v[sl, :].T).astype(BF),
                "woT": np.ascontiguousarray(w_o[:, sl].T).astype(BF),
                "bq": np.ascontiguousarray(b_q[sl].reshape(HALF, 1)),
                "bk": np.ascontiguousarray(b_k[sl].reshape(HALF, 1)),
            }
        )

    res = run_bass_kernel_spmd(nc, in_maps, core_ids=list(range(N_CORES)))

    const_row = (b_v[None, :] @ w_o.T + b_o[None, :]).astype(np.float32)
    out = np.empty((B, T, D), dtype=np.float32)
    for b in range(B):
        out[b] = np.asarray(res.results[2 * b]["partial"], dtype=np.float32)
        out[b] += np.asarray(res.results[2 * b + 1]["partial"], dtype=np.float32)
        out[b] += const_row
    return out
